# revision 5
# baseline (speedup 1.0000x reference)
"""Trainium2 Bass kernel for pair/all-pairs scoring.

Math (same decomposition as the reference):
    s1 = sent_feat @ W[:D],  s2 = sent_feat @ W[D:]
    all_score[i, j]  = s1[i] + s2[j] + b
    pair_score[e, k] = (s1[edge[e,0]] + b) + s2[edge[e,k+1]]

Sharding (8 cores, row-parallel):
    core c owns sent_feat rows [c*1024, (c+1)*1024) and edge rows likewise.
    Each core computes its local s1/s2 slab (DVE multiply+reduce), the slabs
    are exchanged with two 8-core AllGathers (4 KiB each), then each core
    computes its [1024, 8192] slab of all_score (DVE outer-sum against a
    partition-broadcast copy of s2) and its 1024 rows of pair_score (GPSIMD
    ap_gather from the replicated [s2_full ; s1_full+b] vector).
"""

import sys

sys.path.insert(0, "/opt/trn_rl_repo")

import numpy as np

N = 8192  # sentences
D = 1024  # feature dim
E = 8192  # edge rows
K = 16  # 1 center + 15 neighbors
NCORES = 8
NLOC = N // NCORES  # 1024 rows per core
ELOC = E // NCORES  # 1024 edge rows per core
P = 128
NT = NLOC // P  # 8 feature tiles per core
JT = 512  # all_score column tile
NJ = N // JT  # 16 column tiles
NJR = NJ // NCORES  # 2 column tiles per rank block
EG = ELOC // P  # 8 gpsimd groups of 128 edges
NIDX = ELOC * K // EG  # 2048 gather indices per group

_CACHE = {}


def _build(stage=4):
    """stage: 1=matvec only, 2=+exchange, 3=+main loop, 4=+pair (full)."""
    from concourse import bacc, mybir, tile

    f32 = mybir.dt.float32
    i16 = mybir.dt.int16

    nc = bacc.Bacc("TRN2", target_bir_lowering=False, debug=False, num_devices=NCORES)

    feat = nc.dram_tensor("feat", [NLOC, D], f32, kind="ExternalInput")
    wrep = nc.dram_tensor("wrep", [P, 2 * D + 1], f32, kind="ExternalInput")
    eidx = nc.dram_tensor("eidx", [16 * EG, NIDX // 16], i16, kind="ExternalInput")
    ident_in = nc.dram_tensor("ident", [P, P], f32, kind="ExternalInput")
    all_out = nc.dram_tensor("all_out", [NLOC, N], f32, kind="ExternalOutput")
    pair_out = nc.dram_tensor("pair_out", [ELOC, K - 1], f32, kind="ExternalOutput")

    AX = mybir.AxisListType.X
    ADD = mybir.AluOpType.add

    with tile.TileContext(nc) as tc:
        with (
            tc.tile_pool(name="const", bufs=1) as constp,
            tc.tile_pool(name="featp", bufs=3) as featp,
            tc.tile_pool(name="prod", bufs=2) as prodp,
            tc.tile_pool(name="svec", bufs=1) as svecp,
            tc.tile_pool(name="big", bufs=1) as bigp,
            tc.tile_pool(name="outp", bufs=6) as outp,
            tc.tile_pool(name="gth", bufs=1) as gthp,
            tc.tile_pool(name="ps", bufs=2, space="PSUM") as psp,
            tc.tile_pool(name="dram", bufs=1, space="DRAM") as dramp,
        ):
            wsb = constp.tile([P, 2 * D + 1], f32)
            nc.sync.dma_start(wsb[:], wrep[:])
            idxsb = constp.tile([16 * EG, NIDX // 16], i16)
            nc.sync.dma_start(idxsb[:], eidx[:])
            ident = constp.tile([P, P], f32)
            nc.sync.dma_start(ident[:], ident_in[:])

            # ---- local matvec: s1/s2 columns, one [128, 1] slice per tile ----
            s1p = svecp.tile([P, NT], f32)
            s2p = svecp.tile([P, NT], f32)
            for i in range(NT):
                ft = featp.tile([P, D], f32)
                nc.sync.dma_start(ft[:], feat[i * P : (i + 1) * P, :])
                p2 = prodp.tile([P, D], f32, tag="p2")
                nc.vector.tensor_mul(p2[:], ft[:], wsb[:, D : 2 * D])
                nc.vector.tensor_reduce(s2p[:, i : i + 1], p2[:], axis=AX, op=ADD)
                p1 = prodp.tile([P, D], f32, tag="p1")
                nc.vector.tensor_mul(p1[:], ft[:], wsb[:, 0:D])
                nc.vector.tensor_reduce(s1p[:, i : i + 1], p1[:], axis=AX, op=ADD)
            # fold the bias into s1
            nc.vector.tensor_add(
                s1p[:], s1p[:], wsb[:, 2 * D : 2 * D + 1].broadcast_to([P, NT])
            )

            # ---- exchange: transpose to free-major, AllGather s2 then s1 ----
            if stage >= 2:
                pt2 = psp.tile([NT, P], f32, tag="pt")
                nc.tensor.transpose(pt2[:], s2p[:], ident[:])
                s2T = svecp.tile([NT, P], f32)
                nc.vector.tensor_copy(s2T[:], pt2[:])
                cc1_in = dramp.tile([NT, P], f32, tag="cc1i")
                nc.sync.dma_start(cc1_in[:], s2T[:])
                cc1_out = dramp.tile([N], f32, tag="cc1o")
                nc.gpsimd.collective_compute(
                    "AllGather",
                    mybir.AluOpType.bypass,
                    replica_groups=[list(range(NCORES))],
                    ins=[cc1_in.opt()],
                    outs=[cc1_out.opt()],
                )

                pt1 = psp.tile([NT, P], f32, tag="pt")
                nc.tensor.transpose(pt1[:], s1p[:], ident[:])
                s1T = svecp.tile([NT, P], f32)
                nc.vector.tensor_copy(s1T[:], pt1[:])
                cc2_in = dramp.tile([NT, P], f32, tag="cc2i")
                nc.sync.dma_start(cc2_in[:], s1T[:])
                cc2_out = dramp.tile([N], f32, tag="cc2o")
                nc.gpsimd.collective_compute(
                    "AllGather",
                    mybir.AluOpType.bypass,
                    replica_groups=[list(range(NCORES))],
                    ins=[cc2_in.opt()],
                    outs=[cc2_out.opt()],
                )

            # ---- all_score main loop, rank-interleaved with s2 broadcast ----
            if stage >= 3:
                s2b = bigp.tile([P, N], f32, tag="s2b")
                nc.sync.dma_start(s2b[0:1, :], cc1_out.opt())
                for r in range(NCORES):
                    nc.gpsimd.partition_broadcast(
                        s2b[:, r * NLOC : (r + 1) * NLOC],
                        s2b[0:1, r * NLOC : (r + 1) * NLOC],
                    )
                    for i in range(NT):
                        for jj in range(NJR):
                            j = r * NJR + jj
                            ot = outp.tile([P, JT], f32)
                            nc.vector.tensor_add(
                                ot[:],
                                s2b[:, j * JT : (j + 1) * JT],
                                s1p[:, i : i + 1].broadcast_to([P, JT]),
                            )
                            nc.sync.dma_start(
                                all_out[i * P : (i + 1) * P, j * JT : (j + 1) * JT],
                                ot[:],
                            )

            # ---- pair_score: replicated gather source [s2_full ; s1_full+b] ----
            if stage >= 4:
                scg = bigp.tile([P, 2 * N], f32, tag="scg")
                nc.sync.dma_start(scg[0:1, 0:N], cc1_out.opt())
                nc.sync.dma_start(scg[0:1, N : 2 * N], cc2_out.opt())
                nc.gpsimd.partition_broadcast(scg[:, :], scg[0:1, :])
                g = gthp.tile([16 * EG, NIDX], f32)
                nc.gpsimd.ap_gather(
                    g[:],
                    scg[0 : 16 * EG, :],
                    idxsb[:],
                    channels=16 * EG,
                    num_elems=2 * N,
                    d=1,
                    num_idxs=NIDX,
                )
                # one partition per 16-partition group holds that group's values
                scratch = dramp.tile([EG, NIDX], f32, tag="scr")
                for gi in range(EG):
                    nc.sync.dma_start(
                        scratch[gi : gi + 1, :], g[16 * gi : 16 * gi + 1, :]
                    )
                pg = gthp.tile([P, P], f32)
                nc.sync.dma_start(
                    pg[:], scratch.opt().rearrange("a (b f) -> (a b) f", f=P)
                )
                pgv = pg[:].rearrange("p (m k) -> p m k", k=K)
                pairt = gthp.tile([P, (P // K) * (K - 1)], f32)
                pairtv = pairt[:].rearrange("p (m k) -> p m k", k=K - 1)
                nc.vector.tensor_add(
                    pairtv,
                    pgv[:, :, 1:K],
                    pgv[:, :, 0:1].broadcast_to([P, P // K, K - 1]),
                )
                nc.sync.dma_start(
                    pair_out.ap().rearrange("(a b) k -> a (b k)", b=P // K), pairt[:]
                )

    nc.compile()
    return nc


def get_nc():
    if "nc" not in _CACHE:
        _CACHE["nc"] = _build()
    return _CACHE["nc"]


def host_inputs(sent_feat, W, b, edge):
    """Build the per-core input maps (pure marshalling: slab slicing, weight
    replication, and int16 gather-offset precomputation)."""
    sent_feat = np.ascontiguousarray(np.asarray(sent_feat, dtype=np.float32))
    W = np.asarray(W, dtype=np.float32).reshape(2 * D)
    bval = np.float32(np.asarray(b, dtype=np.float32).reshape(-1)[0])
    edge = np.asarray(edge).astype(np.int64).reshape(E, K)

    wrow = np.empty(2 * D + 1, dtype=np.float32)
    wrow[: 2 * D] = W
    wrow[2 * D] = bval
    wrep = np.ascontiguousarray(np.broadcast_to(wrow, (P, 2 * D + 1)))

    ident = np.eye(P, dtype=np.float32)

    # gather offsets into [s2_full (N) ; s1_full+b (N)]
    off = edge.copy()
    off[:, 0] += N  # center scores live in the s1 half
    in_maps = []
    for c in range(NCORES):
        loc = off[c * ELOC : (c + 1) * ELOC]  # [1024, 16]
        idx16 = (
            loc.reshape(EG, P, K).transpose(0, 2, 1).reshape(16 * EG, NIDX // 16)
        ).astype(np.int16)
        in_maps.append(
            {
                "feat": sent_feat[c * NLOC : (c + 1) * NLOC],
                "wrep": wrep,
                "eidx": np.ascontiguousarray(idx16),
                "ident": ident,
            }
        )
    return in_maps


def kernel(sent_feat, W, b, edge):
    from concourse.bass_utils import run_bass_kernel_spmd

    nc = get_nc()
    in_maps = host_inputs(sent_feat, W, b, edge)
    res = run_bass_kernel_spmd(nc, in_maps, list(range(NCORES)))
    all_score = np.concatenate(
        [np.asarray(res.results[c]["all_out"]) for c in range(NCORES)], axis=0
    )
    pair_score = np.concatenate(
        [np.asarray(res.results[c]["pair_out"]) for c in range(NCORES)], axis=0
    )
    return pair_score, all_score


# revision 7
# speedup vs baseline: 1.2703x; 1.2703x over previous
"""Trainium2 Bass kernel for pair/all-pairs scoring.

Math (same decomposition as the reference):
    s1 = sent_feat @ W[:D],  s2 = sent_feat @ W[D:]
    all_score[i, j]  = s1[i] + s2[j] + b
    pair_score[e, k] = (s1[edge[e,0]] + b) + s2[edge[e,k+1]]

Sharding (8 cores, row-parallel):
    core c owns sent_feat rows [c*1024, (c+1)*1024) and edge rows likewise.
    Each core computes its local s1/s2 slab (fused DVE multiply+reduce), one
    8-core AllGather shares all slabs (16 KiB), the shared row is replicated
    across partitions with a K=1 ones-matmul on the TensorEngine, and each
    core then computes its [1024, 8192] slab of all_score (outer-sum split
    between the Vector and Scalar engines) and its 1024 rows of pair_score
    (GPSIMD ap_gather from the replicated score row).
"""

import sys

sys.path.insert(0, "/opt/trn_rl_repo")

import numpy as np

N = 8192  # sentences
D = 1024  # feature dim
E = 8192  # edge rows
K = 16  # 1 center + 15 neighbors
NCORES = 8
NLOC = N // NCORES  # 1024 rows per core
ELOC = E // NCORES  # 1024 edge rows per core
P = 128
NT = NLOC // P  # 8 feature tiles per core
JT = 1024  # all_score column tile (= one rank block)
EG = ELOC // P  # 8 gpsimd groups of 128 edges
NIDX = ELOC * K // EG  # 2048 gather indices per group
BC = 512  # ones-matmul broadcast chunk (PE moving-free limit)

_CACHE = {}


def _build(stage=4):
    """stage: 1=matvec only, 2=+exchange, 3=+main loop, 4=+pair (full)."""
    from concourse import bacc, mybir, tile

    f32 = mybir.dt.float32
    i16 = mybir.dt.int16

    nc = bacc.Bacc("TRN2", target_bir_lowering=False, debug=False, num_devices=NCORES)

    feat = nc.dram_tensor("feat", [NLOC, D], f32, kind="ExternalInput")
    wrep = nc.dram_tensor("wrep", [P, 2 * D + 1], f32, kind="ExternalInput")
    eidx = nc.dram_tensor("eidx", [16 * EG, NIDX // 16], i16, kind="ExternalInput")
    ident_in = nc.dram_tensor("ident", [P, P], f32, kind="ExternalInput")
    all_out = nc.dram_tensor("all_out", [NLOC, N], f32, kind="ExternalOutput")
    pair_out = nc.dram_tensor("pair_out", [ELOC, K - 1], f32, kind="ExternalOutput")

    MUL = mybir.AluOpType.mult

    with tile.TileContext(nc) as tc:
        with (
            tc.tile_pool(name="const", bufs=1) as constp,
            tc.tile_pool(name="featp", bufs=3) as featp,
            tc.tile_pool(name="prod", bufs=2) as prodp,
            tc.tile_pool(name="svec", bufs=1) as svecp,
            tc.tile_pool(name="big", bufs=1) as bigp,
            tc.tile_pool(name="outp", bufs=5) as outp,
            tc.tile_pool(name="gth", bufs=1) as gthp,
            tc.tile_pool(name="ps", bufs=4, space="PSUM") as psp,
            tc.tile_pool(name="pt", bufs=2, space="PSUM") as ptp,
            tc.tile_pool(name="dram", bufs=1, space="DRAM") as dramp,
        ):
            wsb = constp.tile([P, 2 * D + 1], f32)
            nc.sync.dma_start(wsb[:], wrep[:])
            idxsb = constp.tile([16 * EG, NIDX // 16], i16)
            nc.sync.dma_start(idxsb[:], eidx[:])
            ident = constp.tile([P, P], f32)
            nc.sync.dma_start(ident[:], ident_in[:])
            ones = constp.tile([1, P], f32)
            nc.vector.memset(ones[:], 1.0)

            # ---- local matvec: fused multiply+accumulate along free dim ----
            s1p = svecp.tile([P, NT], f32)
            s2p = svecp.tile([P, NT], f32)
            for i in range(NT):
                ft = featp.tile([P, D], f32)
                nc.sync.dma_start(ft[:], feat[i * P : (i + 1) * P, :])
                p2 = prodp.tile([P, D], f32, tag="prod")
                nc.vector.scalar_tensor_tensor(
                    p2[:], ft[:], 1.0, wsb[:, D : 2 * D], MUL, MUL,
                    accum_out=s2p[:, i : i + 1],
                )
                p1 = prodp.tile([P, D], f32, tag="prod")
                nc.vector.scalar_tensor_tensor(
                    p1[:], ft[:], 1.0, wsb[:, 0:D], MUL, MUL,
                    accum_out=s1p[:, i : i + 1],
                )
            # fold the bias into s1
            nc.vector.tensor_add(
                s1p[:], s1p[:], wsb[:, 2 * D : 2 * D + 1].broadcast_to([P, NT])
            )

            # ---- exchange: transpose to free-major, single AllGather ----
            pt2 = ptp.tile([NT, P], f32, tag="pt")
            nc.tensor.transpose(pt2[:], s2p[:], ident[:])
            s2T = svecp.tile([NT, P], f32)
            nc.vector.tensor_copy(s2T[:], pt2[:])
            pt1 = ptp.tile([NT, P], f32, tag="pt")
            nc.tensor.transpose(pt1[:], s1p[:], ident[:])
            s1T = svecp.tile([NT, P], f32)
            nc.vector.tensor_copy(s1T[:], pt1[:])

            # per-rank contribution: [s2_own (1024) ; s1_own+b (1024)]
            cc_in = dramp.tile([2 * NT, P], f32, tag="cci")
            nc.sync.dma_start(cc_in[0:NT, :], s2T[:])
            nc.sync.dma_start(cc_in[NT : 2 * NT, :], s1T[:])
            cc_out = dramp.tile([2 * N], f32, tag="cco")
            nc.gpsimd.collective_compute(
                "AllGather",
                mybir.AluOpType.bypass,
                replica_groups=[list(range(NCORES))],
                ins=[cc_in.opt()],
                outs=[cc_out.opt()],
            )

            # ---- replicate the shared row across partitions (PE ones-matmul),
            # ---- then the outer-sum main loop split across DVE and ACT
            scg = bigp.tile([P, 2 * N], f32, tag="scg")
            nc.sync.dma_start(scg[0:1, :], cc_out.opt())
            eng = 0
            for r in range(NCORES):
                base = r * 2 * NLOC
                for cpart in range(2 * NLOC // BC):  # replicate the whole rank block
                    lo = base + cpart * BC
                    pb = psp.tile([P, BC], f32, tag="bc")
                    nc.tensor.matmul(
                        pb[:], ones[:], scg[0:1, lo : lo + BC], start=True, stop=True
                    )
                    if cpart % 2 == 0:
                        nc.vector.tensor_copy(scg[:, lo : lo + BC], pb[:])
                    else:
                        nc.scalar.copy(scg[:, lo : lo + BC], pb[:])
                for i in range(NT):
                    ot = outp.tile([P, JT], f32)
                    if eng % 2 == 0:
                        nc.vector.tensor_add(
                            ot[:],
                            scg[:, base : base + NLOC],
                            s1p[:, i : i + 1].broadcast_to([P, JT]),
                        )
                    else:
                        nc.scalar.add(ot[:], scg[:, base : base + NLOC], s1p[:, i : i + 1])
                    eng += 1
                    nc.sync.dma_start(
                        all_out[i * P : (i + 1) * P, r * JT : (r + 1) * JT], ot[:]
                    )

            # ---- pair_score: gpsimd gather from the replicated score row ----
            g = gthp.tile([16 * EG, NIDX], f32)
            nc.gpsimd.ap_gather(
                g[:],
                scg[0 : 16 * EG, :],
                idxsb[:],
                channels=16 * EG,
                num_elems=2 * N,
                d=1,
                num_idxs=NIDX,
            )
            # one partition per 16-partition group holds that group's values
            scratch = dramp.tile([EG, NIDX], f32, tag="scr")
            for gi in range(EG):
                nc.sync.dma_start(scratch[gi : gi + 1, :], g[16 * gi : 16 * gi + 1, :])
            pg = gthp.tile([P, P], f32)
            nc.sync.dma_start(pg[:], scratch.opt().rearrange("a (b f) -> (a b) f", f=P))
            pgv = pg[:].rearrange("p (m k) -> p m k", k=K)
            pairt = gthp.tile([P, (P // K) * (K - 1)], f32)
            pairtv = pairt[:].rearrange("p (m k) -> p m k", k=K - 1)
            nc.vector.tensor_add(
                pairtv,
                pgv[:, :, 1:K],
                pgv[:, :, 0:1].broadcast_to([P, P // K, K - 1]),
            )
            nc.sync.dma_start(
                pair_out.ap().rearrange("(a b) k -> a (b k)", b=P // K), pairt[:]
            )

    return _finish(nc)


def _finish(nc):
    nc.compile()
    return nc


def get_nc():
    if "nc" not in _CACHE:
        _CACHE["nc"] = _build()
    return _CACHE["nc"]


def host_inputs(sent_feat, W, b, edge):
    """Build the per-core input maps (pure marshalling: slab slicing, weight
    replication, and int16 gather-offset precomputation)."""
    sent_feat = np.ascontiguousarray(np.asarray(sent_feat, dtype=np.float32))
    W = np.asarray(W, dtype=np.float32).reshape(2 * D)
    bval = np.float32(np.asarray(b, dtype=np.float32).reshape(-1)[0])
    edge = np.asarray(edge).astype(np.int64).reshape(E, K)

    wrow = np.empty(2 * D + 1, dtype=np.float32)
    wrow[: 2 * D] = W
    wrow[2 * D] = bval
    wrep = np.ascontiguousarray(np.broadcast_to(wrow, (P, 2 * D + 1)))

    ident = np.eye(P, dtype=np.float32)

    # gather offsets into the AllGather layout:
    #   rank r block: [s2_own (NLOC) ; s1_own+b (NLOC)] at r*2*NLOC
    rank = edge // NLOC
    within = edge % NLOC
    off = rank * (2 * NLOC) + within  # s2 half
    off[:, 0] += NLOC  # center score lives in the s1 half
    in_maps = []
    for c in range(NCORES):
        loc = off[c * ELOC : (c + 1) * ELOC]  # [1024, 16]
        idx16 = (
            loc.reshape(EG, P, K).transpose(0, 2, 1).reshape(16 * EG, NIDX // 16)
        ).astype(np.int16)
        in_maps.append(
            {
                "feat": sent_feat[c * NLOC : (c + 1) * NLOC],
                "wrep": wrep,
                "eidx": np.ascontiguousarray(idx16),
                "ident": ident,
            }
        )
    return in_maps


def kernel(sent_feat, W, b, edge):
    from concourse.bass_utils import run_bass_kernel_spmd

    nc = get_nc()
    in_maps = host_inputs(sent_feat, W, b, edge)
    res = run_bass_kernel_spmd(nc, in_maps, list(range(NCORES)))
    all_score = np.concatenate(
        [np.asarray(res.results[c]["all_out"]) for c in range(NCORES)], axis=0
    )
    pair_score = np.concatenate(
        [np.asarray(res.results[c]["pair_out"]) for c in range(NCORES)], axis=0
    )
    return pair_score, all_score
